# revision 19
# baseline (speedup 1.0000x reference)
"""Trainium2 Bass kernel for nn_CCL_50740743635433 (class-collapsed CCL loss).

Math: with C=64 classes, pos_centroid[i] == class_centroid[labels[i]], so the
reference's 8192x8192 distance matrix collapses to 8192x64:
  class_sum[c,:]  = sum_{i: lab_i==c} preds[i,:]      (one-hot matmul)
  cent[c,:]       = class_sum[c,:] / count[c]
  sq[i,c]         = |p_i|^2 + |cent_c|^2 - 2 p_i.cent_c
  pos[i]          = sqrt(max(sq[i, lab_i],0));  neg[i] = sqrt(max(min_{c != lab_i} sq[i,c],0))
  loss            = mean softplus(pos - neg + 0.2)

Device/host split (v2): the device computes, per own row i,
  gneg[i] = min_c (|c_c|^2 - 2 p_i.c_c + 1e10*onehot[i,c])
  gpos[i] = |c_lab|^2 - 2 p_i.c_lab
(|p_i|^2 is constant across c, so it commutes with the min and moves to the
host along with clamp/sqrt/softplus/mean.)  This removes Exp/Ln/Relu/Square
activations (zero act-table loads), the on-device Newton sqrt, and the count
reciprocal chain (counts/-2/cnt/(1/cnt)^2/absent masks are label-only, so the
host precomputes them).

Memory strategy: every core gets the FULL preds, host-cast to bf16 (halves
HBM traffic; the matmuls ran in bf16 already) and host-rearranged chunk-major
[128, 64*128] so each partition's DMA run is 2KB contiguous (128 descriptors
per group instead of 2048).  Cross-core collectives measured ~78us on this
rig (dispatch skew) — fully replicated compute stays.

Per-core work: core c evaluates distances for its own 1024 rows only and
returns [128, 16] (gneg/gpos interleaved per chunk); the host assembles the
loss.
"""

import sys

sys.path.insert(0, "/opt/trn_rl_repo")

import numpy as np

import concourse.bacc as bacc
import concourse.bass_utils as bass_utils
import concourse.mybir as mybir
import concourse.tile as tile

N = 8192
D = 128
C = 64
N_CORES = 8
ROWS_PER_CORE = N // N_CORES          # 1024
CHUNKS = N // 128                     # 64 chunks of 128 rows
OWN_CHUNKS = ROWS_PER_CORE // 128     # 8 chunks per core
GROUPS = 4
G = CHUNKS // GROUPS                  # 16 chunks per DMA group
ALPHA = 0.2
BIG = 1e10
HUGE = 1e20

f32 = mybir.dt.float32
bf16 = mybir.dt.bfloat16
Alu = mybir.AluOpType
Act = mybir.ActivationFunctionType
Ax = mybir.AxisListType

_compiled = None
last_results = None


def _build():
    import ml_dtypes

    nc = bacc.Bacc(
        "TRN2",
        target_bir_lowering=False,
        debug=False,
        enable_asserts=True,
        num_devices=N_CORES,
    )

    # inputs (host-prepped layouts; see kernel())
    preds_d = nc.dram_tensor("preds_bf", [128, CHUNKS * D], bf16, kind="ExternalInput")
    mypreds_d = nc.dram_tensor(
        "my_preds_bf", [128, OWN_CHUNKS * D], bf16, kind="ExternalInput"
    )
    # two packed bf16 blobs. lblob (tiny, first): 0:64 labels chunk-major |
    # 64:72 my labels | 72:136 iota row bcast.  cblob2: 0:128 identity |
    # 128 ones col | 129:193 stacked diag(-2/cnt) pair (cs_bf @ M =
    # -2*cent^T) | row0 193:257 absent bias | row0 257:385 ones row
    LB = 136
    CB = 385
    lblob_d = nc.dram_tensor("lblob", [128, LB], bf16, kind="ExternalInput")
    cblob_d = nc.dram_tensor("cblob2", [128, CB], bf16, kind="ExternalInput")
    out_d = nc.dram_tensor("out", [128, 2 * OWN_CHUNKS], f32, kind="ExternalOutput")

    with tile.TileContext(nc) as tc:
        with (
            tc.tile_pool(name="cst", bufs=1) as cst,
            tc.tile_pool(name="big", bufs=1) as bigp,
            tc.tile_pool(name="wrk", bufs=1) as wrk,
            tc.tile_pool(name="pacc", bufs=1, space="PSUM") as pacc,
            tc.tile_pool(name="pt", bufs=3, space="PSUM") as pt,
            tc.tile_pool(name="pct", bufs=1, space="PSUM") as pct,
            tc.tile_pool(name="pg", bufs=2, space="PSUM") as pg,
        ):
            # all preds groups immediately on the sync queue: within a
            # queue descriptors complete in order, so group g lands fully
            # before g+1 and phase A streams behind the DMA
            preds_re = preds_d.ap().rearrange("p (j d) -> p j d", d=D)
            psb_g = []
            for g in range(GROUPS):
                pf = bigp.tile([128, G, D], bf16, name=f"psb{g}", tag=f"psb{g}")
                nc.sync.dma_start(pf[:], preds_re[:, g * G : (g + 1) * G, :])
                psb_g.append(pf)

            # scalar queue: tiny label blob (gates one-hots), own shard,
            # then the rest of the consts
            lblob = cst.tile([128, LB], bf16)
            nc.scalar.dma_start(lblob[:], lblob_d.ap())
            lsb = lblob[:, 0:64]
            mylsb = lblob[:, 64:72]
            iota_b = lblob[:, 72:136].rearrange("p (j c) -> p j c", j=1)

            osb = wrk.tile([128, OWN_CHUNKS, D], bf16)
            nc.scalar.dma_start(
                osb[:],
                mypreds_d.ap().rearrange("p (j d) -> p j d", d=D),
            )
            cblob = cst.tile([128, CB], bf16)
            nc.scalar.dma_start(cblob[:], cblob_d.ap())
            identb = cblob[:, 0:128]
            onescol_b = cblob[:, 128:129]
            m_diag = cblob[:, 129:193]
            ab_row = cblob[0:1, 193:257]
            onesrow_b = cblob[0:1, 257:385]

            # ---- one-hots per group (gate phase A) ----
            oh_g = []
            for g in range(GROUPS):
                t = bigp.tile([128, G, C], bf16, name=f"oh{g}", tag=f"oh{g}")
                nc.vector.tensor_tensor(
                    t[:],
                    lsb[:, g * G : (g + 1) * G].to_broadcast((128, G, C)),
                    iota_b.to_broadcast((128, G, C)),
                    Alu.is_equal,
                )
                oh_g.append(t)

            # own-chunk masks (vector, early, off critical path)
            ohm = wrk.tile([128, OWN_CHUNKS, C], bf16)
            nc.vector.tensor_tensor(
                ohm[:],
                mylsb[:].to_broadcast((128, OWN_CHUNKS, C)),
                iota_b.to_broadcast((128, OWN_CHUNKS, C)),
                Alu.is_equal,
            )
            ohinv = wrk.tile([128, OWN_CHUNKS, 2, C], f32)
            nc.vector.tensor_scalar(
                ohinv[:, :, 0, :], ohm[:], BIG, None, Alu.mult
            )
            nc.vector.tensor_scalar(
                ohinv[:, :, 1, :], ohm[:], -BIG, BIG, Alu.mult, Alu.add
            )

            # ---- own-chunk transposes first (PE is idle while the first
            #      preds group streams in) ----
            pts_bf = wrk.tile([128, OWN_CHUNKS, D], bf16)
            for j in range(OWN_CHUNKS):
                ptb = pt.tile([128, 128], bf16, name=f"ptb{j}", tag="ptb")
                nc.tensor.transpose(ptb[:], osb[:, j, :], identb)
                nc.scalar.activation(pts_bf[:, j, :], ptb[:], Act.Copy)

            # ---- phase A: class sums, even/odd col-packed ----
            psum_cs2 = pacc.tile([128, D], f32)
            for j in range(CHUNKS):
                g, jj = j // G, j % G
                half = j % 2
                nc.tensor.matmul(
                    psum_cs2[64 * half : 64 * half + 64, :],
                    oh_g[g][:, jj, :],
                    psb_g[g][:, jj, :],
                    start=(j < 2),
                    stop=(j >= CHUNKS - 2),
                    tile_position=(0, 64 * half),
                    skip_group_check=True,
                )

            # ---- centroid chain ----
            # cs_bf [c2, d] @ M [c2, c] -> psum_ct2 [d, c] = -2 * cent^T
            cs_bf = wrk.tile([128, D], bf16)
            nc.scalar.activation(cs_bf[:], psum_cs2[:], Act.Copy)
            psum_ct2 = pct.tile([128, C], f32)
            nc.tensor.matmul(psum_ct2[:], cs_bf[:], m_diag)
            centT2_bf = wrk.tile([128, C], bf16)
            nc.scalar.activation(centT2_bf[:], psum_ct2[:], Act.Copy)
            # |c|^2 = sum_d (centT2 * 0.5)^2
            sq2_bf = wrk.tile([128, C], bf16)
            nc.scalar.activation(sq2_bf[:], psum_ct2[:], Act.Square, scale=0.5)
            psum_s = pct.tile([1, C], f32, name="psum_s", tag="ps")
            nc.tensor.matmul(psum_s[:], onescol_b, sq2_bf[:])
            csqr_bf = wrk.tile([1, C], bf16)
            nc.vector.tensor_tensor(csqr_bf[:], psum_s[:], ab_row, Alu.add)

            # ---- phase F: per own chunk g = -2 p.c + |c|^2, masked mins ----
            pnsq = wrk.tile([128, 2 * OWN_CHUNKS], f32)
            for pp in range(OWN_CHUNKS // 2):
                psum_pg = pg.tile(
                    [128, 2, C], f32, name=f"pg{pp}", tag="g"
                )
                for u in range(2):
                    j = 2 * pp + u
                    nc.tensor.matmul(
                        psum_pg[:, u, :], pts_bf[:, j, :], centT2_bf[:],
                        start=True, stop=False,
                    )
                    nc.tensor.matmul(
                        psum_pg[:, u, :], onesrow_b, csqr_bf[:],
                        start=False, stop=True, skip_group_check=True,
                    )
                pair = wrk.tile(
                    [128, 2, 2, C], f32, name=f"pair{pp}", tag=f"pair{pp}"
                )
                nc.vector.tensor_tensor(
                    pair[:],
                    psum_pg[:].rearrange("p j (u c) -> p j u c", u=1).to_broadcast(
                        (128, 2, 2, C)
                    ),
                    ohinv[:, 2 * pp : 2 * pp + 2, :, :],
                    Alu.add,
                )
                nc.vector.tensor_reduce(
                    pnsq[:, 4 * pp : 4 * pp + 4], pair[:], Ax.X, Alu.min
                )

            nc.sync.dma_start(out_d.ap(), pnsq[:])

    nc.compile()
    return nc


def _get_compiled():
    global _compiled
    if _compiled is None:
        _compiled = _build()
    return _compiled


def _chunk_major(x, n_chunks):
    # x [n_chunks*128, ...] -> [128, n_chunks, ...] -> [128, n_chunks*...]
    y = x.reshape(n_chunks, 128, -1).transpose(1, 0, 2).reshape(128, -1)
    return np.ascontiguousarray(y)


def kernel(preds, labels, _trace=False):
    import ml_dtypes

    preds = np.asarray(preds, dtype=np.float32)
    lab = np.asarray(labels).astype(np.int64)
    assert preds.shape == (N, D) and lab.shape == (N,)

    preds_bf = preds.astype(ml_dtypes.bfloat16)
    preds_cm = _chunk_major(preds_bf, CHUNKS)
    lab_f = lab.astype(np.float32)

    # packed per-core const blobs (see _build for the layouts)
    cnt = np.bincount(lab, minlength=C).astype(np.float64)
    safe = np.maximum(cnt, 1.0)
    lbase = np.zeros((128, 136), dtype=np.float32)
    lbase[:, 0:64] = _chunk_major(lab_f, CHUNKS)
    lbase[:, 72:136] = np.arange(C, dtype=np.float32)[None, :]
    cb2 = np.zeros((128, 385), dtype=np.float32)
    cb2[:, 0:128] = np.eye(128, dtype=np.float32)
    cb2[:, 128] = 1.0
    cb2[0:64, 129:193] = np.diag(-2.0 / safe)
    cb2[64:128, 129:193] = np.diag(-2.0 / safe)
    cb2[0, 193:257] = np.where(cnt == 0, HUGE, 0.0)
    cb2[0, 257:385] = 1.0
    cb2 = cb2.astype(ml_dtypes.bfloat16)

    nc = _get_compiled()
    in_maps = []
    for c in range(N_CORES):
        r0, r1 = c * ROWS_PER_CORE, (c + 1) * ROWS_PER_CORE
        lb = lbase.copy()
        lb[:, 64:72] = _chunk_major(lab_f[r0:r1], OWN_CHUNKS)
        in_maps.append(
            {
                "preds_bf": preds_cm,
                "my_preds_bf": _chunk_major(preds_bf[r0:r1], OWN_CHUNKS),
                "lblob": lb.astype(ml_dtypes.bfloat16),
                "cblob2": cb2,
            }
        )

    res = bass_utils.run_bass_kernel_spmd(
        nc, in_maps, core_ids=list(range(N_CORES)), trace=_trace
    )
    global last_results
    last_results = res

    # host epilogue: add |p|^2, clamp, sqrt, softplus, mean
    psq = (preds_bf.astype(np.float32) ** 2).sum(axis=1)  # [N]
    total = 0.0
    for c in range(N_CORES):
        o = res.results[c]["out"]  # [128, 16] (gneg, gpos per chunk)
        r0 = c * ROWS_PER_CORE
        # row p, col 2j   = gneg for global row r0 + j*128 + p
        # row p, col 2j+1 = gpos
        gneg = o[:, 0::2].T.reshape(-1)  # [8*128] chunk-major -> rows
        gpos = o[:, 1::2].T.reshape(-1)
        myq = psq[r0 : r0 + ROWS_PER_CORE].reshape(OWN_CHUNKS, 128).reshape(-1)
        negsq = np.maximum(myq + gneg, 0.0)
        possq = np.maximum(myq + gpos, 0.0)
        x = np.sqrt(possq) - np.sqrt(negsq) + ALPHA
        total += np.sum(np.log1p(np.exp(x)))
    return np.float32(total / N)


# revision 22
# speedup vs baseline: 1.1742x; 1.1742x over previous
"""Trainium2 Bass kernel for nn_CCL_50740743635433 (class-collapsed CCL loss).

Math: with C=64 classes, pos_centroid[i] == class_centroid[labels[i]], so the
reference's 8192x8192 distance matrix collapses to 8192x64:
  class_sum[c,:]  = sum_{i: lab_i==c} preds[i,:]      (one-hot matmul)
  cent[c,:]       = class_sum[c,:] / count[c]
  sq[i,c]         = |p_i|^2 + |cent_c|^2 - 2 p_i.cent_c
  pos[i]          = sqrt(max(sq[i, lab_i],0));  neg[i] = sqrt(max(min_{c != lab_i} sq[i,c],0))
  loss            = mean softplus(pos - neg + 0.2)

Device/host split (v2): the device computes, per own row i,
  gneg[i] = min_c (|c_c|^2 - 2 p_i.c_c + 1e10*onehot[i,c])
  gpos[i] = |c_lab|^2 - 2 p_i.c_lab
(|p_i|^2 is constant across c, so it commutes with the min and moves to the
host along with clamp/sqrt/softplus/mean.)  This removes Exp/Ln/Relu/Square
activations (zero act-table loads), the on-device Newton sqrt, and the count
reciprocal chain (counts/-2/cnt/(1/cnt)^2/absent masks are label-only, so the
host precomputes them).

Memory strategy: every core gets the FULL preds, host-cast to bf16 (halves
HBM traffic; the matmuls ran in bf16 already) and host-rearranged chunk-major
[128, 64*128] so each partition's DMA run is 2KB contiguous (128 descriptors
per group instead of 2048).  Cross-core collectives measured ~78us on this
rig (dispatch skew) — fully replicated compute stays.

Per-core work: core c evaluates distances for its own 1024 rows only and
returns [128, 16] (gneg/gpos interleaved per chunk); the host assembles the
loss.
"""

import sys

sys.path.insert(0, "/opt/trn_rl_repo")

import numpy as np

import concourse.bacc as bacc
import concourse.bass_utils as bass_utils
import concourse.mybir as mybir
import concourse.tile as tile

N = 8192
D = 128
C = 64
N_CORES = 8
ROWS_PER_CORE = N // N_CORES          # 1024
CHUNKS = N // 128                     # 64 chunks of 128 rows
OWN_CHUNKS = ROWS_PER_CORE // 128     # 8 chunks per core
GROUPS = 4
G = CHUNKS // GROUPS                  # 16 chunks per DMA group
ALPHA = 0.2
BIG = 1e10
HUGE = 1e20

f32 = mybir.dt.float32
bf16 = mybir.dt.bfloat16
Alu = mybir.AluOpType
Act = mybir.ActivationFunctionType
Ax = mybir.AxisListType

_compiled = None
last_results = None


def _build():
    import ml_dtypes

    nc = bacc.Bacc(
        "TRN2",
        target_bir_lowering=False,
        debug=False,
        enable_asserts=True,
        num_devices=N_CORES,
    )

    # inputs (host-prepped layouts; see kernel())
    preds_d = nc.dram_tensor("preds_bf", [128, CHUNKS * D], bf16, kind="ExternalInput")
    mypreds_d = nc.dram_tensor(
        "my_preds_bf", [128, OWN_CHUNKS * D], bf16, kind="ExternalInput"
    )
    # one packed bf16 blob for all small inputs (single DMA):
    # 0:64 labels chunk-major | 64:72 my labels | 72:200 identity |
    # 200:264 iota row bcast | 264 ones col | 265:329 stacked diag(-2/cnt)
    # pair (cs_bf @ M = -2*cent^T) | row0 329:393 absent bias | row0
    # 393:521 ones row
    CB = 521
    cblob_d = nc.dram_tensor("cblob", [128, CB], bf16, kind="ExternalInput")
    out_d = nc.dram_tensor("out", [128, 2 * OWN_CHUNKS], f32, kind="ExternalOutput")

    with tile.TileContext(nc) as tc:
        with (
            tc.tile_pool(name="cst", bufs=1) as cst,
            tc.tile_pool(name="big", bufs=1) as bigp,
            tc.tile_pool(name="wrk", bufs=1) as wrk,
            tc.tile_pool(name="pacc", bufs=1, space="PSUM") as pacc,
            tc.tile_pool(name="pt", bufs=3, space="PSUM") as pt,
            tc.tile_pool(name="pct", bufs=1, space="PSUM") as pct,
            tc.tile_pool(name="pg", bufs=2, space="PSUM") as pg,
        ):
            # ---- packed small-input blob first on the sync queue ----
            cblob = cst.tile([128, CB], bf16)
            nc.sync.dma_start(cblob[:], cblob_d.ap())
            lsb = cblob[:, 0:64]
            mylsb = cblob[:, 64:72]
            identb = cblob[:, 72:200]
            iota_b = cblob[:, 200:264].rearrange("p (j c) -> p j c", j=1)
            onescol_b = cblob[:, 264:265]
            m_diag = cblob[:, 265:329]
            ab_row = cblob[0:1, 329:393]
            onesrow_b = cblob[0:1, 393:521]

            # own shard on the scalar queue (transposes early)
            osb = wrk.tile([128, OWN_CHUNKS, D], bf16)
            nc.scalar.dma_start(
                osb[:],
                mypreds_d.ap().rearrange("p (j d) -> p j d", d=D),
            )
            # all preds groups on ONE queue: within a queue descriptors
            # complete in order, so group g lands fully before g+1 and
            # phase A streams behind the DMA instead of waiting for a
            # round-robin tail
            preds_re = preds_d.ap().rearrange("p (j d) -> p j d", d=D)
            psb_g = []
            for g in range(GROUPS):
                pf = bigp.tile([128, G, D], bf16, name=f"psb{g}", tag=f"psb{g}")
                nc.sync.dma_start(pf[:], preds_re[:, g * G : (g + 1) * G, :])
                psb_g.append(pf)

            # ---- one-hots per group (gate phase A) ----
            oh_g = []
            for g in range(GROUPS):
                t = bigp.tile([128, G, C], bf16, name=f"oh{g}", tag=f"oh{g}")
                nc.vector.tensor_tensor(
                    t[:],
                    lsb[:, g * G : (g + 1) * G].to_broadcast((128, G, C)),
                    iota_b.to_broadcast((128, G, C)),
                    Alu.is_equal,
                )
                oh_g.append(t)

            # own-chunk masks (vector, early, off critical path)
            ohm = wrk.tile([128, OWN_CHUNKS, C], bf16)
            nc.vector.tensor_tensor(
                ohm[:],
                mylsb[:].to_broadcast((128, OWN_CHUNKS, C)),
                iota_b.to_broadcast((128, OWN_CHUNKS, C)),
                Alu.is_equal,
            )
            ohinv = wrk.tile([128, OWN_CHUNKS, 2, C], f32)
            nc.vector.tensor_scalar(
                ohinv[:, :, 0, :], ohm[:], BIG, None, Alu.mult
            )
            nc.vector.tensor_scalar(
                ohinv[:, :, 1, :], ohm[:], -BIG, BIG, Alu.mult, Alu.add
            )

            # ---- own-chunk transposes first (PE is idle while the first
            #      preds group streams in) ----
            pts_bf = wrk.tile([128, OWN_CHUNKS, D], bf16)
            for j in range(OWN_CHUNKS):
                ptb = pt.tile([128, 128], bf16, name=f"ptb{j}", tag="ptb")
                nc.tensor.transpose(ptb[:], osb[:, j, :], identb)
                nc.scalar.activation(pts_bf[:, j, :], ptb[:], Act.Copy)

            # ---- phase A: class sums, even/odd col-packed ----
            psum_cs2 = pacc.tile([128, D], f32)
            for j in range(CHUNKS):
                g, jj = j // G, j % G
                half = j % 2
                nc.tensor.matmul(
                    psum_cs2[64 * half : 64 * half + 64, :],
                    oh_g[g][:, jj, :],
                    psb_g[g][:, jj, :],
                    start=(j < 2),
                    stop=(j >= CHUNKS - 2),
                    tile_position=(0, 64 * half),
                    skip_group_check=True,
                )

            # ---- centroid chain ----
            # cs_bf [c2, d] @ M [c2, c] -> psum_ct2 [d, c] = -2 * cent^T
            cs_bf = wrk.tile([128, D], bf16)
            nc.scalar.activation(cs_bf[:], psum_cs2[:], Act.Copy)
            psum_ct2 = pct.tile([128, C], f32)
            nc.tensor.matmul(psum_ct2[:], cs_bf[:], m_diag)
            centT2_bf = wrk.tile([128, C], bf16)
            nc.scalar.activation(centT2_bf[:], psum_ct2[:], Act.Copy)
            # |c|^2 = sum_d (centT2 * 0.5)^2
            sq2_bf = wrk.tile([128, C], bf16)
            nc.scalar.activation(sq2_bf[:], psum_ct2[:], Act.Square, scale=0.5)
            psum_s = pct.tile([1, C], f32, name="psum_s", tag="ps")
            nc.tensor.matmul(psum_s[:], onescol_b, sq2_bf[:])
            csqr_bf = wrk.tile([1, C], bf16)
            nc.vector.tensor_tensor(csqr_bf[:], psum_s[:], ab_row, Alu.add)

            # ---- phase F: per own chunk g = -2 p.c + |c|^2, masked mins ----
            pnsq = wrk.tile([128, 2 * OWN_CHUNKS], f32)
            for pp in range(OWN_CHUNKS // 2):
                psum_pg = pg.tile(
                    [128, 2, C], f32, name=f"pg{pp}", tag="g"
                )
                for u in range(2):
                    j = 2 * pp + u
                    nc.tensor.matmul(
                        psum_pg[:, u, :], pts_bf[:, j, :], centT2_bf[:],
                        start=True, stop=False,
                    )
                    nc.tensor.matmul(
                        psum_pg[:, u, :], onesrow_b, csqr_bf[:],
                        start=False, stop=True, skip_group_check=True,
                    )
                pair = wrk.tile(
                    [128, 2, 2, C], f32, name=f"pair{pp}", tag=f"pair{pp}"
                )
                nc.vector.tensor_tensor(
                    pair[:],
                    psum_pg[:].rearrange("p j (u c) -> p j u c", u=1).to_broadcast(
                        (128, 2, 2, C)
                    ),
                    ohinv[:, 2 * pp : 2 * pp + 2, :, :],
                    Alu.add,
                )
                nc.vector.tensor_reduce(
                    pnsq[:, 4 * pp : 4 * pp + 4], pair[:], Ax.X, Alu.min
                )

            nc.sync.dma_start(out_d.ap(), pnsq[:])

    nc.compile()
    return nc


def _get_compiled():
    global _compiled
    if _compiled is None:
        _compiled = _build()
    return _compiled


def _chunk_major(x, n_chunks):
    # x [n_chunks*128, ...] -> [128, n_chunks, ...] -> [128, n_chunks*...]
    y = x.reshape(n_chunks, 128, -1).transpose(1, 0, 2).reshape(128, -1)
    return np.ascontiguousarray(y)


def kernel(preds, labels, _trace=False):
    import ml_dtypes

    preds = np.asarray(preds, dtype=np.float32)
    lab = np.asarray(labels).astype(np.int64)
    assert preds.shape == (N, D) and lab.shape == (N,)

    preds_bf = preds.astype(ml_dtypes.bfloat16)
    preds_cm = _chunk_major(preds_bf, CHUNKS)
    lab_f = lab.astype(np.float32)

    # packed per-core const blob (see _build for the layout)
    cnt = np.bincount(lab, minlength=C).astype(np.float64)
    safe = np.maximum(cnt, 1.0)
    base = np.zeros((128, 521), dtype=np.float32)
    base[:, 0:64] = _chunk_major(lab_f, CHUNKS)
    base[:, 72:200] = np.eye(128, dtype=np.float32)
    base[:, 200:264] = np.arange(C, dtype=np.float32)[None, :]
    base[:, 264] = 1.0
    base[0:64, 265:329] = np.diag(-2.0 / safe)
    base[64:128, 265:329] = np.diag(-2.0 / safe)
    base[0, 329:393] = np.where(cnt == 0, HUGE, 0.0)
    base[0, 393:521] = 1.0

    nc = _get_compiled()
    in_maps = []
    for c in range(N_CORES):
        r0, r1 = c * ROWS_PER_CORE, (c + 1) * ROWS_PER_CORE
        cb = base.copy()
        cb[:, 64:72] = _chunk_major(lab_f[r0:r1], OWN_CHUNKS)
        in_maps.append(
            {
                "preds_bf": preds_cm,
                "my_preds_bf": _chunk_major(preds_bf[r0:r1], OWN_CHUNKS),
                "cblob": cb.astype(ml_dtypes.bfloat16),
            }
        )

    res = bass_utils.run_bass_kernel_spmd(
        nc, in_maps, core_ids=list(range(N_CORES)), trace=_trace
    )
    global last_results
    last_results = res

    # host epilogue: add |p|^2, clamp, sqrt, softplus, mean
    psq = (preds_bf.astype(np.float32) ** 2).sum(axis=1)  # [N]
    total = 0.0
    for c in range(N_CORES):
        o = res.results[c]["out"]  # [128, 16] (gneg, gpos per chunk)
        r0 = c * ROWS_PER_CORE
        # row p, col 2j   = gneg for global row r0 + j*128 + p
        # row p, col 2j+1 = gpos
        gneg = o[:, 0::2].T.reshape(-1)  # [8*128] chunk-major -> rows
        gpos = o[:, 1::2].T.reshape(-1)
        myq = psq[r0 : r0 + ROWS_PER_CORE].reshape(OWN_CHUNKS, 128).reshape(-1)
        negsq = np.maximum(myq + gneg, 0.0)
        possq = np.maximum(myq + gpos, 0.0)
        x = np.sqrt(possq) - np.sqrt(negsq) + ALPHA
        total += np.sum(np.log1p(np.exp(x)))
    return np.float32(total / N)
